# revision 20
# baseline (speedup 1.0000x reference)
"""ARIMA(2,1,2) residual (eps) kernel for 8 TRN2 NeuronCores.

Math
----
The reference computes, for t in [2, T) (T = len(y) - 1):

    yd[t]  = y[t+1] - y[t]
    ar[t]  = phi0*y[t] + phi1*y[t-1]
    eps[t] = yd[t] - mu - ar[t] - theta0*eps[t-1] - theta1*eps[t-2]

with eps[0] = eps[1] = 0, output out[o] = eps[o+2] for o < T-2 and
out[T-2] = out[T-1] = 0.

This is a constant-coefficient order-2 linear recurrence driven by
c[t] = y[t+1] - (1+phi0)*y[t] - phi1*y[t-1] - mu.  Its impulse response
w (w[0]=1, w[1]=-theta0, w[n]=-theta0*w[n-1]-theta1*w[n-2]) decays
geometrically (|roots| ~ 0.2 for the given coefficient scale), so eps is
numerically a short FIR of c, i.e. a short FIR of y:

    out[o] = sum_j G[j] * y[o + 3 - j],   G = conv(w[:K], [1, -(1+phi0), -phi1])

with K chosen so the dropped tail is < 1e-9.  The FIR is evaluated on
the TensorEngine as a banded matmul: lay the series time-major down the
128 partitions (columns = consecutive 128-sample blocks) and then

    OUT[:, c] = B0 @ Y[:, c] + B1 @ Y[:, c-1]

where B0/B1 are the [128,128] intra/inter-block bands of G.  Each core
processes a contiguous 1/8 slice of the output with a 1-column halo --
embarrassingly parallel, no collectives.  A tiny additive correction
fixes the first 128 outputs (recurrence warm-up) and a mask zeroes the
final two outputs.

Performance notes
-----------------
- I/O in fp16: y ~ N(0,1) so fp16 (10-bit mantissa) costs ~2e-4 relative
  error while halving HBM traffic (the kernel is memory-bound).
  Accumulation stays fp32 in PSUM.
- Each dma_start costs ~0.6us of sequencer dispatch; input DMAs all go
  on the sync queue IN ORDER (FIFO completion = dependency order; fanning
  out across queues round-robins the SDMA engines and was measured
  worse), outputs on the scalar queue.
- Dummy matmuls on scratch warm the PE's HAM clock gate (1.2 -> 2.4 GHz)
  while the first input chunk is in flight.
- PSUM is used as 4 two-bank tensors so PSUM->SBUF copies move 1024
  columns per DVE op (amortizes the ~300ns per-op overhead).
"""

import numpy as np

import concourse.bass as bass
from concourse import mybir
from concourse.bass_utils import run_bass_kernel_spmd

NCORES = 8
N = 4194305
T = N - 1  # 4194304 outputs
S = T // NCORES  # 524288 outputs per core
C = S // 128  # 4096 output columns per core
CB = 8  # matmul blocks per core (one PSUM bank each)
BW = C // CB  # 512 columns per block
PG = 4  # psum groups (2 banks each); copies move PW columns at once
PW = C // PG  # 1024
N_WARMUP = 11  # dummy matmuls to warm the PE clock gate (~4.2us cold)

# input y chunks, all on the sync queue in order (after the bands DMA)
YSPLIT = [(0, 513), (513, 1537), (1537, 2561), (2561, 4097)]
# tensor-engine waits emitted before block b (sync order: bands, c0..c3)
BLOCK_WAITS = {0: 32, 1: 48, 3: 64, 5: 80}

_CACHE = {}


def _build_bass():
    f16 = mybir.dt.float16
    f32 = mybir.dt.float32
    nc = bass.Bass()
    yt = nc.declare_dram_parameter("yt", [128, C + 1], f16, isOutput=False)
    # bands: cols [0:128) = B0^T, [128:256) = B1^T, 256 = corr, 257 = mask
    bands = nc.declare_dram_parameter("bands", [128, 258], f16, isOutput=False)
    out = nc.declare_dram_parameter("out", [128, C], f16, isOutput=True)

    from contextlib import ExitStack

    with ExitStack() as ctx:
        y_sb = ctx.enter_context(nc.sbuf_tensor("y_sb", [128, C + 1], f16))
        o_sb = ctx.enter_context(nc.sbuf_tensor("o_sb", [128, C], f16))
        k_sb = ctx.enter_context(nc.sbuf_tensor("k_sb", [128, 258], f16))
        w_sb = ctx.enter_context(nc.sbuf_tensor("w_sb", [128, BW], f16))
        psums = [
            ctx.enter_context(nc.psum_tensor("ps%d" % g, [128, PW], f32))
            for g in range(PG)
        ]
        sems = {}
        for name in ("inS", "mm", "cp", "outd"):
            sems[name] = ctx.enter_context(nc.semaphore(name + "_sem"))
        block = ctx.enter_context(nc.Block())

        b0_ap = k_sb[:, 0:128]
        b1_ap = k_sb[:, 128:256]
        corr_ap = k_sb[:, 256:257]
        mask_ap = k_sb[:, 257:258]

        def psum_blk(b):
            g, h = divmod(b, CB // PG)
            return psums[g][:, h * BW : (h + 1) * BW]

        @block.sync
        def _(sync):
            lo, hi = YSPLIT[0]
            sync.dma_start(out=y_sb[:, lo:hi], in_=yt[:, lo:hi]).then_inc(
                sems["inS"], 16
            )
            sync.dma_start(out=k_sb[:], in_=bands[:]).then_inc(sems["inS"], 16)
            for lo, hi in YSPLIT[1:]:
                sync.dma_start(out=y_sb[:, lo:hi], in_=yt[:, lo:hi]).then_inc(
                    sems["inS"], 16
                )
            sync.wait_ge(sems["outd"], 16 * (PG + 1))

        @block.tensor
        def _(tensor):
            # Warm up the PE clock gate while DMAs land.  w_sb is never
            # written: the dummy results (possibly NaN) go to a PSUM
            # region the real block 0 clears with start=True.
            for _ in range(N_WARMUP):
                tensor.matmul(
                    psums[0][:, 0:BW],
                    w_sb[:, 0:128],
                    w_sb[:],
                    start=True,
                    stop=True,
                    skip_group_check=True,
                )
            for b in range(CB):
                if b in BLOCK_WAITS:
                    tensor.wait_ge(sems["inS"], BLOCK_WAITS[b])
                tensor.matmul(
                    psum_blk(b),
                    b1_ap,
                    y_sb[:, b * BW : b * BW + BW],
                    start=True,
                    stop=False,
                    skip_group_check=True,
                )
                tensor.matmul(
                    psum_blk(b),
                    b0_ap,
                    y_sb[:, b * BW + 1 : b * BW + BW + 1],
                    start=False,
                    stop=True,
                    skip_group_check=True,
                ).then_inc(sems["mm"], 1)

        @block.vector
        def _(vector):
            # NOTE: back-to-back DVE ops do NOT interlock RAW through SBUF
            # (no drain between ops), so the corr/mask fixups read straight
            # from PSUM and write columns disjoint from the bulk copy.
            with nc.allow_low_precision(reason="fp16 output is intentional"):
                for g in range(PG - 1):
                    vector.wait_ge(sems["mm"], 2 * (g + 1))
                    if g == 0:
                        vector.tensor_add(o_sb[:, 0:1], psums[0][:, 0:1], corr_ap)
                        ins = vector.tensor_copy(o_sb[:, 1:PW], psums[0][:, 1:PW])
                    else:
                        ins = vector.tensor_copy(
                            o_sb[:, g * PW : (g + 1) * PW], psums[g][:]
                        )
                    ins.then_inc(sems["cp"], 1)
                # last psum group split in two so the final output DMA
                # (which ends the kernel) covers only BW columns
                g = PG - 1
                vector.wait_ge(sems["mm"], CB - 1)
                vector.tensor_copy(
                    o_sb[:, g * PW : g * PW + BW], psums[g][:, 0:BW]
                ).then_inc(sems["cp"], 1)
                vector.wait_ge(sems["mm"], CB)
                vector.tensor_mul(
                    o_sb[:, C - 1 : C], psums[g][:, PW - 1 : PW], mask_ap
                )
                vector.tensor_copy(
                    o_sb[:, g * PW + BW : C - 1], psums[g][:, BW : PW - 1]
                ).then_inc(sems["cp"], 1)

        @block.scalar
        def _(scalar):
            ochunks = [
                (0, PW),
                (PW, 2 * PW),
                (2 * PW, 3 * PW),
                (3 * PW, 3 * PW + BW),
                (3 * PW + BW, C),
            ]
            for g, (lo, hi) in enumerate(ochunks):
                scalar.wait_ge(sems["cp"], g + 1)
                scalar.dma_start(
                    out=out[:, lo:hi], in_=o_sb[:, lo:hi]
                ).then_inc(sems["outd"], 16)

    return nc


def _host_prep(y, phi, theta, mu):
    """All small host-side constant prep (f64): FIR taps, band matrices,
    warm-up correction, tail mask, and the per-core input layout."""
    y = np.asarray(y, dtype=np.float32)
    p0, p1 = float(phi[0]), float(phi[1])
    t0, t1 = float(theta[0]), float(theta[1])
    m = float(mu[0])

    # Fold a nonzero mu into a constant shift of y (exact when the AR
    # stencil gain is nonzero; mu is zeros for this problem's inputs).
    if m != 0.0 and abs(p0 + p1) > 1e-12:
        y = y - np.float32(m / (-(p0 + p1)))

    # Impulse response of 1/(1 + t0 z + t1 z^2), truncated adaptively.
    wmax = 126
    w = np.zeros(wmax + 2, dtype=np.float64)
    w[0] = 1.0
    w[1] = -t0
    for n in range(2, wmax + 2):
        w[n] = -t0 * w[n - 1] - t1 * w[n - 2]
    K = wmax
    for k in range(4, wmax):
        if abs(w[k]) + abs(w[k + 1]) < 1e-10:
            K = k
            break
    w = w[:K]
    a = np.array([1.0, -(1.0 + p0), -p1], dtype=np.float64)
    G = np.convolve(w, a)  # length J = K + 2
    J = len(G)
    assert J <= 128

    # Band matrices (transposed for the matmul's lhsT operand).
    p_idx = np.arange(128)[None, :]
    q_idx = np.arange(128)[:, None]
    d0 = p_idx - q_idx
    d1 = d0 + 128
    b0t_np = np.where((d0 >= 0) & (d0 < J), G[np.clip(d0, 0, J - 1)], 0.0)
    b1t_np = np.where((d1 >= 0) & (d1 < J), G[np.clip(d1, 0, J - 1)], 0.0)

    # The device computes the FIR with fp16-rounded taps and fp16 y;
    # mirror that rounding in the warm-up correction's FIR reference.
    b0t_16 = b0t_np.astype(np.float16)
    b1t_16 = b1t_np.astype(np.float16)

    # Warm-up correction for the first 128 outputs: true recurrence
    # minus what the FIR computes (both in f64, fp16 tap rounding).
    Gq = np.array(
        [float(np.float16(G[j])) for j in range(J)], dtype=np.float64
    )
    y64 = y[: 140 + J].astype(np.float64)
    eps = np.zeros(132, dtype=np.float64)  # eps[t] for t in [0, 132)
    for t in range(2, 132):
        c = y64[t + 1] - (1.0 + p0) * y64[t] - p1 * y64[t - 1] - m
        eps[t] = c - t0 * eps[t - 1] - t1 * eps[t - 2]
    yq = np.array(
        [float(np.float16(v)) for v in y64], dtype=np.float64
    )
    y_ext = np.concatenate([np.zeros(J, dtype=np.float64), yq])
    fir0 = np.zeros(128, dtype=np.float64)
    for o in range(128):
        acc = 0.0
        for j in range(J):
            acc += Gq[j] * y_ext[J + o + 3 - j]
        fir0[o] = acc
    corr0 = eps[2:130] - fir0

    bands0 = np.zeros((128, 258), dtype=np.float16)
    bands0[:, 0:128] = b0t_16
    bands0[:, 128:256] = b1t_16
    bands0[:, 256] = corr0.astype(np.float16)
    bands0[:, 257] = 1.0
    bands_mid = bands0.copy()
    bands_mid[:, 256] = 0.0
    bands_last = bands_mid.copy()
    bands_last[126, 257] = 0.0
    bands_last[127, 257] = 0.0

    # Per-core inputs: yt[q, cc] = y_ext[m*S + (cc-1)*128 + q + 3]
    # (zero-padded outside [0, N)).
    y16 = y.astype(np.float16)
    ypad = np.pad(y16, (128, 160))
    in_maps = []
    for mcore in range(NCORES):
        flat = ypad[mcore * S + 3 : mcore * S + 3 + (C + 1) * 128]
        yt_np = np.ascontiguousarray(flat.reshape(C + 1, 128).T)
        if mcore == 0:
            kn = bands0
        elif mcore == NCORES - 1:
            kn = bands_last
        else:
            kn = bands_mid
        in_maps.append({"yt": yt_np, "bands": kn})
    return in_maps


def kernel(y, phi, theta, mu):
    assert y.shape == (N,)
    if "nc" not in _CACHE:
        _CACHE["nc"] = _build_bass()
    nc = _CACHE["nc"]
    in_maps = _host_prep(y, phi, theta, mu)
    res = run_bass_kernel_spmd(nc, in_maps, list(range(NCORES)))
    out = np.empty(T, dtype=np.float32)
    for mcore in range(NCORES):
        blk = res.results[mcore]["out"]  # [128, C] fp16
        out[mcore * S : (mcore + 1) * S] = blk.T.reshape(-1).astype(np.float32)
    return out


# revision 23
# speedup vs baseline: 1.0258x; 1.0258x over previous
"""ARIMA(2,1,2) residual (eps) kernel for 8 TRN2 NeuronCores.

Math
----
The reference computes, for t in [2, T) (T = len(y) - 1):

    yd[t]  = y[t+1] - y[t]
    ar[t]  = phi0*y[t] + phi1*y[t-1]
    eps[t] = yd[t] - mu - ar[t] - theta0*eps[t-1] - theta1*eps[t-2]

with eps[0] = eps[1] = 0, output out[o] = eps[o+2] for o < T-2 and
out[T-2] = out[T-1] = 0.

This is a constant-coefficient order-2 linear recurrence driven by
c[t] = y[t+1] - (1+phi0)*y[t] - phi1*y[t-1] - mu.  Its impulse response
w (w[0]=1, w[1]=-theta0, w[n]=-theta0*w[n-1]-theta1*w[n-2]) decays
geometrically (|roots| ~ 0.2 for the given coefficient scale), so eps is
numerically a short FIR of c, i.e. a short FIR of y:

    out[o] = sum_j G[j] * y[o + 3 - j],   G = conv(w[:K], [1, -(1+phi0), -phi1])

with K chosen so the dropped tail is < 1e-9.  The FIR is evaluated on
the TensorEngine as a banded matmul: lay the series time-major down the
128 partitions (columns = consecutive 128-sample blocks) and then

    OUT[:, c] = B0 @ Y[:, c] + B1 @ Y[:, c-1]

where B0/B1 are the [128,128] intra/inter-block bands of G.  Each core
processes a contiguous 1/8 slice of the output with a 1-column halo --
embarrassingly parallel, no collectives.  A tiny additive correction
fixes the first 128 outputs (recurrence warm-up) and a mask zeroes the
final two outputs.

Performance notes
-----------------
- I/O in fp16: y ~ N(0,1) so fp16 (10-bit mantissa) costs ~2e-4 relative
  error while halving HBM traffic (the kernel is memory-bound).
  Accumulation stays fp32 in PSUM.
- Each dma_start costs ~0.6us of sequencer dispatch; input DMAs all go
  on the sync queue IN ORDER (FIFO completion = dependency order; fanning
  out across queues round-robins the SDMA engines and was measured
  worse), outputs on the scalar queue.
- Dummy matmuls on scratch warm the PE's HAM clock gate (1.2 -> 2.4 GHz)
  while the first input chunk is in flight.
- PSUM is used as 4 two-bank tensors so PSUM->SBUF copies move 1024
  columns per DVE op (amortizes the ~300ns per-op overhead).
"""

import numpy as np

import concourse.bass as bass
from concourse import mybir
from concourse.bass_utils import run_bass_kernel_spmd

NCORES = 8
N = 4194305
T = N - 1  # 4194304 outputs
S = T // NCORES  # 524288 outputs per core
C = S // 128  # 4096 output columns per core
CB = 8  # matmul blocks per core (one PSUM bank each)
BW = C // CB  # 512 columns per block
PG = 4  # psum groups (2 banks each); copies move PW columns at once
PW = C // PG  # 1024
N_WARMUP = 11  # dummy matmuls to warm the PE clock gate (~4.2us cold)

# input y chunks, all on the sync queue in order (after the bands DMA)
YSPLIT = [(0, 513), (513, 1537), (1537, 2561), (2561, 4097)]
# tensor-engine waits emitted before block b (sync order: bands, c0..c3)
BLOCK_WAITS = {0: 32, 1: 48, 3: 64, 5: 80}

_CACHE = {}


def _build_bass():
    f16 = mybir.dt.float16
    f32 = mybir.dt.float32
    nc = bass.Bass()
    yt = nc.declare_dram_parameter("yt", [128, C + 1], f16, isOutput=False)
    # bands: cols [0:128) = B0^T, [128:256) = B1^T, 256 = corr, 257 = mask
    bands = nc.declare_dram_parameter("bands", [128, 258], f16, isOutput=False)
    out = nc.declare_dram_parameter("out", [128, C], f16, isOutput=True)

    from contextlib import ExitStack

    with ExitStack() as ctx:
        y_sb = ctx.enter_context(nc.sbuf_tensor("y_sb", [128, C + 1], f16))
        o_sb = ctx.enter_context(nc.sbuf_tensor("o_sb", [128, C], f16))
        k_sb = ctx.enter_context(nc.sbuf_tensor("k_sb", [128, 258], f16))
        w_sb = ctx.enter_context(nc.sbuf_tensor("w_sb", [128, BW], f16))
        psums = [
            ctx.enter_context(nc.psum_tensor("ps%d" % g, [128, PW], f32))
            for g in range(PG)
        ]
        sems = {}
        for name in ("inS", "mm", "cp", "outd", "wu"):
            sems[name] = ctx.enter_context(nc.semaphore(name + "_sem"))
        block = ctx.enter_context(nc.Block())

        b0_ap = k_sb[:, 0:128]
        b1_ap = k_sb[:, 128:256]
        corr_ap = k_sb[:, 256:257]
        mask_ap = k_sb[:, 257:258]

        def psum_blk(b):
            g, h = divmod(b, CB // PG)
            return psums[g][:, h * BW : (h + 1) * BW]

        @block.sync
        def _(sync):
            lo, hi = YSPLIT[0]
            sync.dma_start(out=y_sb[:, lo:hi], in_=yt[:, lo:hi]).then_inc(
                sems["inS"], 16
            )
            sync.dma_start(out=k_sb[:], in_=bands[:]).then_inc(sems["inS"], 16)
            for lo, hi in YSPLIT[1:]:
                sync.dma_start(out=y_sb[:, lo:hi], in_=yt[:, lo:hi]).then_inc(
                    sems["inS"], 16
                )
            sync.wait_ge(sems["outd"], 16 * (PG + 1))

        @block.tensor
        def _(tensor):
            # Warm up the PE clock gate while DMAs land.  w_sb is zeroed
            # by the vector engine first (uninitialized SBUF can hold
            # fp16 NaN patterns, and we don't want those anywhere near
            # the PE/PSUM state).
            tensor.wait_ge(sems["wu"], 1)
            for _ in range(N_WARMUP):
                tensor.matmul(
                    psums[0][:, 0:BW],
                    w_sb[:, 0:128],
                    w_sb[:],
                    start=True,
                    stop=True,
                    skip_group_check=True,
                )
            for b in range(CB):
                if b in BLOCK_WAITS:
                    tensor.wait_ge(sems["inS"], BLOCK_WAITS[b])
                tensor.matmul(
                    psum_blk(b),
                    b1_ap,
                    y_sb[:, b * BW : b * BW + BW],
                    start=True,
                    stop=False,
                    skip_group_check=True,
                )
                tensor.matmul(
                    psum_blk(b),
                    b0_ap,
                    y_sb[:, b * BW + 1 : b * BW + BW + 1],
                    start=False,
                    stop=True,
                    skip_group_check=True,
                ).then_inc(sems["mm"], 1)

        @block.vector
        def _(vector):
            vector.memset(w_sb[:], 0.0).then_inc(sems["wu"], 1)
            # NOTE: back-to-back DVE ops do NOT interlock RAW through SBUF
            # (no drain between ops), so the corr/mask fixups read straight
            # from PSUM and write columns disjoint from the bulk copy.
            with nc.allow_low_precision(reason="fp16 output is intentional"):
                for g in range(PG - 1):
                    vector.wait_ge(sems["mm"], 2 * (g + 1))
                    if g == 0:
                        vector.tensor_add(o_sb[:, 0:1], psums[0][:, 0:1], corr_ap)
                        ins = vector.tensor_copy(o_sb[:, 1:PW], psums[0][:, 1:PW])
                    else:
                        ins = vector.tensor_copy(
                            o_sb[:, g * PW : (g + 1) * PW], psums[g][:]
                        )
                    ins.then_inc(sems["cp"], 1)
                # last psum group split in two so the final output DMA
                # (which ends the kernel) covers only BW columns
                g = PG - 1
                vector.wait_ge(sems["mm"], CB - 1)
                vector.tensor_copy(
                    o_sb[:, g * PW : g * PW + BW], psums[g][:, 0:BW]
                ).then_inc(sems["cp"], 1)
                vector.wait_ge(sems["mm"], CB)
                vector.tensor_mul(
                    o_sb[:, C - 1 : C], psums[g][:, PW - 1 : PW], mask_ap
                )
                vector.tensor_copy(
                    o_sb[:, g * PW + BW : C - 1], psums[g][:, BW : PW - 1]
                ).then_inc(sems["cp"], 1)

        @block.scalar
        def _(scalar):
            ochunks = [
                (0, PW),
                (PW, 2 * PW),
                (2 * PW, 3 * PW),
                (3 * PW, 3 * PW + BW),
                (3 * PW + BW, C),
            ]
            for g, (lo, hi) in enumerate(ochunks):
                scalar.wait_ge(sems["cp"], g + 1)
                scalar.dma_start(
                    out=out[:, lo:hi], in_=o_sb[:, lo:hi]
                ).then_inc(sems["outd"], 16)

    return nc


def _host_prep(y, phi, theta, mu):
    """All small host-side constant prep (f64): FIR taps, band matrices,
    warm-up correction, tail mask, and the per-core input layout."""
    y = np.asarray(y, dtype=np.float32)
    p0, p1 = float(phi[0]), float(phi[1])
    t0, t1 = float(theta[0]), float(theta[1])
    m = float(mu[0])

    # Fold a nonzero mu into a constant shift of y (exact when the AR
    # stencil gain is nonzero; mu is zeros for this problem's inputs).
    if m != 0.0 and abs(p0 + p1) > 1e-12:
        y = y - np.float32(m / (-(p0 + p1)))

    # Impulse response of 1/(1 + t0 z + t1 z^2), truncated adaptively.
    wmax = 126
    w = np.zeros(wmax + 2, dtype=np.float64)
    w[0] = 1.0
    w[1] = -t0
    for n in range(2, wmax + 2):
        w[n] = -t0 * w[n - 1] - t1 * w[n - 2]
    K = wmax
    for k in range(4, wmax):
        if abs(w[k]) + abs(w[k + 1]) < 1e-10:
            K = k
            break
    w = w[:K]
    a = np.array([1.0, -(1.0 + p0), -p1], dtype=np.float64)
    G = np.convolve(w, a)  # length J = K + 2
    J = len(G)
    assert J <= 128

    # Band matrices (transposed for the matmul's lhsT operand).
    p_idx = np.arange(128)[None, :]
    q_idx = np.arange(128)[:, None]
    d0 = p_idx - q_idx
    d1 = d0 + 128
    b0t_np = np.where((d0 >= 0) & (d0 < J), G[np.clip(d0, 0, J - 1)], 0.0)
    b1t_np = np.where((d1 >= 0) & (d1 < J), G[np.clip(d1, 0, J - 1)], 0.0)

    # The device computes the FIR with fp16-rounded taps and fp16 y;
    # mirror that rounding in the warm-up correction's FIR reference.
    b0t_16 = b0t_np.astype(np.float16)
    b1t_16 = b1t_np.astype(np.float16)

    # Warm-up correction for the first 128 outputs: true recurrence
    # minus what the FIR computes (both in f64, fp16 tap rounding).
    Gq = np.array(
        [float(np.float16(G[j])) for j in range(J)], dtype=np.float64
    )
    y64 = y[: 140 + J].astype(np.float64)
    eps = np.zeros(132, dtype=np.float64)  # eps[t] for t in [0, 132)
    for t in range(2, 132):
        c = y64[t + 1] - (1.0 + p0) * y64[t] - p1 * y64[t - 1] - m
        eps[t] = c - t0 * eps[t - 1] - t1 * eps[t - 2]
    yq = np.array(
        [float(np.float16(v)) for v in y64], dtype=np.float64
    )
    y_ext = np.concatenate([np.zeros(J, dtype=np.float64), yq])
    fir0 = np.zeros(128, dtype=np.float64)
    for o in range(128):
        acc = 0.0
        for j in range(J):
            acc += Gq[j] * y_ext[J + o + 3 - j]
        fir0[o] = acc
    corr0 = eps[2:130] - fir0

    bands0 = np.zeros((128, 258), dtype=np.float16)
    bands0[:, 0:128] = b0t_16
    bands0[:, 128:256] = b1t_16
    bands0[:, 256] = corr0.astype(np.float16)
    bands0[:, 257] = 1.0
    bands_mid = bands0.copy()
    bands_mid[:, 256] = 0.0
    bands_last = bands_mid.copy()
    bands_last[126, 257] = 0.0
    bands_last[127, 257] = 0.0

    # Per-core inputs: yt[q, cc] = y_ext[m*S + (cc-1)*128 + q + 3]
    # (zero-padded outside [0, N)).
    y16 = y.astype(np.float16)
    ypad = np.pad(y16, (128, 160))
    in_maps = []
    for mcore in range(NCORES):
        flat = ypad[mcore * S + 3 : mcore * S + 3 + (C + 1) * 128]
        yt_np = np.ascontiguousarray(flat.reshape(C + 1, 128).T)
        if mcore == 0:
            kn = bands0
        elif mcore == NCORES - 1:
            kn = bands_last
        else:
            kn = bands_mid
        in_maps.append({"yt": yt_np, "bands": kn})
    return in_maps


def kernel(y, phi, theta, mu):
    assert y.shape == (N,)
    if "nc" not in _CACHE:
        _CACHE["nc"] = _build_bass()
    nc = _CACHE["nc"]
    in_maps = _host_prep(y, phi, theta, mu)
    res = run_bass_kernel_spmd(nc, in_maps, list(range(NCORES)))
    out = np.empty(T, dtype=np.float32)
    for mcore in range(NCORES):
        blk = res.results[mcore]["out"]  # [128, C] fp16
        out[mcore * S : (mcore + 1) * S] = blk.T.reshape(-1).astype(np.float32)
    return out
